# revision 19
# baseline (speedup 1.0000x reference)
"""Trainium2 Bass kernel for GatedGraphXBias (gnn_message_passing).

Reference math per iteration (T=2048 notes, E=12 edge types, H=64):
    act[e]  = edge[e].T @ h                      # [T, H]
    a       = sum_e (act[e] + ba[e]) @ W[e] + bw # [T, 3H] -> az|ar|ah
    a      += x @ Win                            # hoisted input projection
    z       = sigmoid(az + h @ Uz)
    r       = sigmoid(ar + h @ Ur)
    h~      = tanh(ah + (r*h) @ Uh)
    h       = (1-z)*h + z*h~

Sharding: sequence-parallel over the note dim T across 8 cores (256 notes
each).  Each core keeps its fp16 edge shard [12, 2048, 256] resident in
SBUF (12 MiB), loaded once per launch (outside the rep loop), with the
full h replicated as matmul weights; per iteration the updated h shard is
AllGather'd in fp16 (ag payload 32 KiB/core).  All f32 tensor-engine
matmuls run as float32r (full-rate fp32 streaming); the big edge matmul
runs in fp16 (1 cycle/row, same rate, half the SBUF).  Gate math runs in
a t-transposed [H, 256] layout so every matmul is layout-natural; the
input projection, U-gate matmuls and all biases fold into the PSUM
accumulation groups.
"""

import sys

sys.path.insert(0, "/opt/trn_rl_repo")

import numpy as np
import concourse.bass as bass
import concourse.mybir as mybir
import concourse.tile as tile
from concourse.bass_utils import run_bass_kernel_spmd
from concourse.masks import make_identity
from concourse.vector_clock import ScopedClock

E, T, H, IN = 12, 2048, 64, 128
M = 8  # cores
TL = T // M  # 256 local notes per core
NCH = T // 128  # 16 contraction chunks of 128 source notes
NPAIR = E // 2  # edge types processed two at a time (n=512 matmuls)
F32 = mybir.dt.float32
F32R = mybir.dt.float32r
F16 = mybir.dt.float16
SIG = mybir.ActivationFunctionType.Sigmoid
TANH = mybir.ActivationFunctionType.Tanh


class SplitDrainTileContext(tile.TileContext):
    """TileContext that limits every instruction to a single sync wait.

    This walrus build rejects >1 sync wait command on an instruction
    (setupSyncWait: "Too many sync wait commands"), so extra waits are
    peeled onto standalone same-engine NoOps emitted just before the
    instruction — semantically identical (the engine stream waits
    sequentially at the same program point)."""

    def _commit_instruction(self, inst, lazy_reg_writes: bool = True):
        si = getattr(inst, "sync_info", None)
        if si is not None and len(si.on_wait) > 1:
            waits = list(si.on_wait)
            inst.sync_info = mybir.SyncInfo(
                on_wait=[waits[-1]], on_update=list(si.on_update)
            )
            for w in waits[:-1]:
                nop = mybir.InstNoOp(
                    name=f"splitwait-{self.nc.next_id()}",
                    sync_info=mybir.SyncInfo(on_wait=[w], on_update=[]),
                    bass_nofuse=True,
                    engine=inst.engine,
                )
                super()._commit_instruction(nop, lazy_reg_writes=False)
        super()._commit_instruction(inst, lazy_reg_writes)

    def _drain_and_barrier(self, tick_clock, wait_clock):
        drain_inst = self.nc.sync.drain()
        wait_clock.add_sem_waits(
            drain_inst.ins, ScopedClock({None: tick_clock.global_clock})
        )
        si = drain_inst.ins.sync_info
        waits = list(si.on_wait) if si is not None else []
        upds = list(si.on_update) if si is not None else []
        if len(waits) > 1:
            drain_inst.ins.sync_info = mybir.SyncInfo(on_wait=waits[:1], on_update=upds)
            for w in waits[1:]:
                nop = self.nc.sync.nop(nofuse=True, hint="split_drain_waits")
                nop.ins.sync_info = mybir.SyncInfo(on_wait=[w], on_update=[])

        self.nc.all_engine_barrier()
        assert self.sems is not None
        popped = self.nc._tile_sem_poison_stack.pop()
        assert popped is self._sem_poison
        self.nc.clear_and_free_semaphores(list(self.sems.allocated().values()))
        self.nc.all_engine_barrier()


def build(
    iteration: int,
    reps: int = 1,
    ablate: frozenset = frozenset(),
    hw_loop: bool = False,
) -> bass.Bass:
    nc = bass.Bass(
        "TRN2",
        target_bir_lowering=False,
        debug=False,
        num_devices=M,
        dynamic_dma_scratch_size=2048,
    )

    # Per-core inputs (host pre-arranged into DMA-friendly layouts):
    #   edge_in : [T, E*TL] f16  row = chunk*128+p (source note), col = e*TL+t
    #   h0s_in  : [128, NCH*H] f16  chunk-major initial h: [p, c*H+j]
    #   hT0_in  : [H, TL]    local initial hidden, transposed
    #   xT_in   : [IN, TL]   local input features, transposed
    #   w_in    : [H, E*3H]  W[e] as lhsT blocks, col = e*192 + k
    #   uzr_in  : [H, 2H] / uh_in : [H, H] / win_in : [IN, 3H]
    #   b{z,r,h}_in : [H, 1] folded biases (bw + sum_e ba[e] @ W[e])
    edge_in = nc.declare_dram_parameter("edge_in", [T, E * TL], F16, isOutput=False)
    h0s_in = nc.declare_dram_parameter("h0s_in", [128, NCH * H], F16, isOutput=False)
    hT0_in = nc.declare_dram_parameter("hT0_in", [H, TL], F32R, isOutput=False)
    xT_in = nc.declare_dram_parameter("xT_in", [IN, TL], F16, isOutput=False)
    w_in = nc.declare_dram_parameter("w_in", [H, E * 3 * H], F16, isOutput=False)
    uzr_in = nc.declare_dram_parameter("uzr_in", [H, 2 * H], F16, isOutput=False)
    uh_in = nc.declare_dram_parameter("uh_in", [H, H], F16, isOutput=False)
    win_in = nc.declare_dram_parameter("win_in", [IN, 3 * H], F16, isOutput=False)
    bz_in = nc.declare_dram_parameter("bz_in", [H, 1], F32, isOutput=False)
    br_in = nc.declare_dram_parameter("br_in", [H, 1], F32, isOutput=False)
    bh_in = nc.declare_dram_parameter("bh_in", [H, 1], F32, isOutput=False)
    h_out = nc.declare_dram_parameter("h_out", [TL, H], F32, isOutput=True)

    with SplitDrainTileContext(nc) as tc:
        with (
            tc.tile_pool(name="edge", bufs=1) as edge_pool,
            tc.tile_pool(name="const", bufs=1) as cpool,
            tc.tile_pool(name="work", bufs=1) as wpool,
            tc.tile_pool(name="psum", bufs=1, space="PSUM") as ppool,
            tc.tile_pool(name="dram", bufs=3, space="DRAM") as dpool,
        ):
            # ---- constants / weights / edge shard (loaded once) ----
            w_sb = cpool.tile([H, E * 3 * H], F16)
            nc.sync.dma_start(out=w_sb[:], in_=w_in[:])
            uzr_sb = cpool.tile([H, 2 * H], F16)
            nc.sync.dma_start(out=uzr_sb[:], in_=uzr_in[:])
            uh_sb = cpool.tile([H, H], F16)
            nc.sync.dma_start(out=uh_sb[:], in_=uh_in[:])
            win_sb = cpool.tile([IN, 3 * H], F16)
            nc.sync.dma_start(out=win_sb[:], in_=win_in[:])
            xT_sb = cpool.tile([IN, TL], F16)
            nc.sync.dma_start(out=xT_sb[:], in_=xT_in[:])
            bz_sb = cpool.tile([H, 1], F32)
            nc.sync.dma_start(out=bz_sb[:], in_=bz_in[:])
            br_sb = cpool.tile([H, 1], F32)
            nc.sync.dma_start(out=br_sb[:], in_=br_in[:])
            bh_sb = cpool.tile([H, 1], F32)
            nc.sync.dma_start(out=bh_sb[:], in_=bh_in[:])
            id64 = cpool.tile([H, H], F32)
            make_identity(nc, id64[:])

            edge_sb = []
            for c in range(NCH):
                et = edge_pool.tile(
                    [128, E * TL], F16, name=f"edge_c{c}", tag=f"edge_c{c}"
                )
                nc.sync.dma_start(out=et[:], in_=edge_in[c * 128 : (c + 1) * 128, :])
                edge_sb.append(et)

            def emit_rep():
                # ---- per-rep state init ----
                h_sb = wpool.tile([128, NCH * H], F16, name="h", tag="h", bufs=3)
                nc.sync.dma_start(out=h_sb[:], in_=h0s_in[:])
                hT_sb = wpool.tile([H, TL], F32R, name="hT", tag="hT", bufs=3)
                nc.sync.dma_start(out=hT_sb[:], in_=hT0_in[:])
                hT16 = wpool.tile([H, TL], F16, tag="hT16", bufs=3)
                nc.vector.tensor_copy(hT16[:], hT_sb[:])

                for it in range(iteration):
                    last = it == iteration - 1

                    # Three PSUM accumulation groups ([64, TL], partition 0):
                    # az, ar, ah.  Each starts with the folded input
                    # projection, absorbs the U-gate matmul, then the 12
                    # per-edge-type W matmuls.
                    az_ps = ppool.tile([H, TL], F32, tag="az", bufs=2)
                    ar_ps = ppool.tile([H, TL], F32, tag="ar")
                    ah_ps = ppool.tile([H, TL], F32, tag="ah")
                    for g, ps in enumerate((az_ps, ar_ps, ah_ps)):
                        nc.tensor.matmul(
                            ps[:],
                            lhsT=win_sb[:, g * H : (g + 1) * H],
                            rhs=xT_sb[:],
                            start=True,
                            stop=False,
                            skip_group_check=True,
                        )
                    for g, ps in enumerate((az_ps, ar_ps)):
                        nc.tensor.matmul(
                            ps[:],
                            lhsT=uzr_sb[:, g * H : (g + 1) * H],
                            rhs=hT16[:],
                            start=False,
                            stop="mm2" in ablate or "mm1" in ablate,
                            skip_group_check=True,
                        )

                    # -- matmul 1 + 2: actT per e-pair, then accumulate.
                    # mm2 for pair p is emitted after mm1 for pair p+1 so the
                    # in-order tensor engine never stalls on the PSUM->SBUF
                    # act copy (it runs under the next pair's mm1).
                    def emit_mm2(pair, act_sb):
                        for k in range(0 if "mm2" in ablate else 2):
                            e = pair * 2 + k
                            for g, ps in enumerate((az_ps, ar_ps, ah_ps)):
                                nc.tensor.matmul(
                                    ps[:],
                                    lhsT=w_sb[
                                        :, e * 3 * H + g * H : e * 3 * H + (g + 1) * H
                                    ],
                                    rhs=act_sb[:, k * TL : (k + 1) * TL],
                                    start=False,
                                    stop=(e == E - 1 and g != 2),
                                    skip_group_check=True,
                                )

                    prev = None
                    for pair in range(0 if "mm1" in ablate else NPAIR):
                        act_ps = ppool.tile([H, 2 * TL], F32, tag="actT", bufs=2)
                        for c in range(NCH):
                            nc.tensor.matmul(
                                act_ps[:],
                                lhsT=h_sb[:, c * H : (c + 1) * H],
                                rhs=edge_sb[c][
                                    :, pair * 2 * TL : (pair + 1) * 2 * TL
                                ],
                                start=(c == 0),
                                stop=(c == NCH - 1),
                                skip_group_check=True,
                            )
                        act_sb = wpool.tile(
                            [H, 2 * TL], F16, name="act", tag="act", bufs=2
                        )
                        nc.vector.tensor_copy(act_sb[:], act_ps[:])
                        if prev is not None:
                            emit_mm2(*prev)
                        prev = (pair, act_sb)
                    if prev is not None:
                        emit_mm2(*prev)

                    # -- gates --
                    # h' = (hT - z*hT) + z*h~ : u = hT - z*hT is computed
                    # while the r -> rh -> uh -> tanh path runs, so only two
                    # vector ops remain after tanh on the critical path.
                    z_sb = wpool.tile([H, TL], F32, tag="z")
                    nc.scalar.activation(z_sb[:], az_ps[:], SIG, bias=bz_sb[:])
                    r_sb = wpool.tile([H, TL], F32, tag="r")
                    nc.scalar.activation(r_sb[:], ar_ps[:], SIG, bias=br_sb[:])
                    rh_sb = wpool.tile([H, TL], F16, tag="rh")
                    nc.vector.tensor_mul(rh_sb[:], r_sb[:], hT_sb[:])
                    nc.tensor.matmul(
                        ah_ps[:],
                        lhsT=uh_sb[:],
                        rhs=rh_sb[:],
                        start=False,
                        stop=True,
                        skip_group_check=True,
                    )
                    zh_sb = wpool.tile([H, TL], F32, tag="zh")
                    nc.vector.tensor_mul(zh_sb[:], z_sb[:], hT_sb[:])
                    u_sb = wpool.tile([H, TL], F32, tag="u")
                    nc.vector.tensor_sub(u_sb[:], hT_sb[:], zh_sb[:])
                    ht_sb = wpool.tile([H, TL], F32, tag="ht")
                    nc.scalar.activation(ht_sb[:], ah_ps[:], TANH, bias=bh_sb[:])

                    zt_sb = wpool.tile([H, TL], F32, tag="zt")
                    nc.vector.tensor_mul(zt_sb[:], z_sb[:], ht_sb[:])
                    hnewT_sb = wpool.tile([H, TL], F32R, tag="hT", bufs=3)
                    nc.vector.tensor_add(hnewT_sb[:], u_sb[:], zt_sb[:])
                    hT16 = wpool.tile([H, TL], F16, tag="hT16", bufs=3)
                    nc.vector.tensor_copy(hT16[:], hnewT_sb[:])

                    if last:
                        hnew_sb = wpool.tile([128, 2 * H], F32R, tag="hnew")
                        for half in range(2):
                            tr_ps = ppool.tile([128, H], F32, tag="tr", bufs=2)
                            nc.tensor.transpose(
                                tr_ps[:],
                                hnewT_sb[:, half * 128 : (half + 1) * 128].bitcast(
                                    F32
                                ),
                                id64[:],
                            )
                            nc.vector.tensor_copy(
                                hnew_sb[:, half * H : (half + 1) * H], tr_ps[:]
                            )
                        nc.sync.dma_start(
                            out=h_out[:].rearrange("(c p) j -> p c j", p=128),
                            in_=hnew_sb[:].bitcast(F32).rearrange(
                                "p (c j) -> p c j", c=2
                            ),
                        )
                    else:
                        # fp16 transpose payload -> DRAM -> AllGather ->
                        # chunk-major h reload
                        hnew16 = wpool.tile([128, 2 * H], F16, tag="hnew16", bufs=3)
                        for half in range(2):
                            tr_ps = ppool.tile([128, H], F32, tag="tr", bufs=2)
                            nc.tensor.transpose(
                                tr_ps[:],
                                hnewT_sb[:, half * 128 : (half + 1) * 128].bitcast(
                                    F32
                                ),
                                id64[:],
                            )
                            nc.vector.tensor_copy(
                                hnew16[:, half * H : (half + 1) * H], tr_ps[:]
                            )
                        if "noex" not in ablate:
                            ag_in = dpool.tile([TL, H], F16, tag="ag_in")
                            nc.sync.dma_start(
                                out=ag_in[:].rearrange("(c p) j -> p c j", p=128),
                                in_=hnew16[:].rearrange("p (c j) -> p c j", c=2),
                            )
                            ag_out = dpool.tile(
                                [T, H], F16, tag="ag_out", addr_space="Shared"
                            )
                            nc.gpsimd.collective_compute(
                                "AllGather",
                                mybir.AluOpType.bypass,
                                replica_groups=[list(range(M))],
                                ins=[ag_in[:]],
                                outs=[ag_out[:]],
                            )
                            h_sb = wpool.tile(
                                [128, NCH * H], F16, name="h", tag="h", bufs=3
                            )
                            nc.sync.dma_start(
                                out=h_sb[:].rearrange("p (c j) -> p c j", c=NCH),
                                in_=ag_out[:].rearrange("(c p) j -> p c j", p=128),
                            )
                        hT_sb = hnewT_sb

            if hw_loop:
                assert "noex" in ablate, "collectives cannot run inside For_i"
                with tc.For_i(0, reps):
                    emit_rep()
            else:
                for _ in range(reps):
                    emit_rep()

    return nc


def _host_prep(input, hidden, edge_matrix, ba, wz_wr_wh, uz_ur, uh, input_wzrh, bw):
    """Pre-arrange full inputs into the per-core DMA layouts."""
    x = np.asarray(input, np.float32)[0]  # [T, IN]
    h0 = np.ascontiguousarray(np.asarray(hidden, np.float32)[0])  # [T, H]
    edge = np.asarray(edge_matrix, np.float32)  # [E, T, T]
    ba = np.asarray(ba, np.float32)
    W = np.asarray(wz_wr_wh, np.float32)  # [E, H, 3H]
    uzr = np.ascontiguousarray(np.asarray(uz_ur, np.float32))
    uh_ = np.ascontiguousarray(np.asarray(uh, np.float32))
    win = np.ascontiguousarray(np.asarray(input_wzrh, np.float32))
    bw = np.asarray(bw, np.float32)

    # folded bias: bw + sum_e ba[e] @ W[e]
    btot = bw + np.einsum("eh,ehk->k", ba, W)  # [3H]
    bz = np.ascontiguousarray(btot[:H].reshape(H, 1))
    br = np.ascontiguousarray(btot[H : 2 * H].reshape(H, 1))
    bh = np.ascontiguousarray(btot[2 * H :].reshape(H, 1))

    # edge shards: shard[m][c*128+p, e*TL+tl] = edge[e, c*128+p, m*TL+tl]
    esh = np.ascontiguousarray(
        edge.reshape(E, NCH, 128, M, TL).transpose(3, 1, 2, 0, 4)
    ).reshape(M, T, E * TL)

    w_flat = np.ascontiguousarray(W.transpose(1, 0, 2)).reshape(H, E * 3 * H)
    # chunk-major h0: h0s[p, c*H+j] = h0[c*128+p, j]
    h0s = np.ascontiguousarray(
        h0.reshape(NCH, 128, H).transpose(1, 0, 2).reshape(128, NCH * H)
    ).astype(np.float16)

    in_maps = []
    for m in range(M):
        xT = np.ascontiguousarray(x[m * TL : (m + 1) * TL, :].T)
        hT0 = np.ascontiguousarray(h0[m * TL : (m + 1) * TL, :].T)
        in_maps.append(
            {
                "edge_in": esh[m].astype(np.float16),
                "h0s_in": h0s,
                "hT0_in": hT0,
                "xT_in": xT.astype(np.float16),
                "w_in": w_flat.astype(np.float16),
                "uzr_in": uzr.astype(np.float16),
                "uh_in": uh_.astype(np.float16),
                "win_in": win.astype(np.float16),
                "bz_in": bz,
                "br_in": br,
                "bh_in": bh,
            }
        )
    return in_maps


_NC_CACHE: dict = {}


def _get_nc(
    iteration: int,
    reps: int = 1,
    ablate: frozenset = frozenset(),
    hw_loop: bool = False,
) -> bass.Bass:
    key = (iteration, reps, ablate, hw_loop)
    if key not in _NC_CACHE:
        _NC_CACHE[key] = build(iteration, reps=reps, ablate=ablate, hw_loop=hw_loop)
    return _NC_CACHE[key]


def kernel(
    input,
    hidden,
    edge_matrix,
    ba,
    wz_wr_wh,
    uz_ur,
    uh,
    input_wzrh,
    bw,
    iteration,
):
    iteration = int(iteration)
    if iteration <= 0:
        return np.asarray(hidden, np.float32).copy()

    nc = _get_nc(iteration)
    in_maps = _host_prep(
        input, hidden, edge_matrix, ba, wz_wr_wh, uz_ur, uh, input_wzrh, bw
    )
    res = run_bass_kernel_spmd(nc, in_maps, list(range(M)))
    out = np.concatenate([res.results[m]["h_out"] for m in range(M)], axis=0)
    return out[None]
